# revision 56
# baseline (speedup 1.0000x reference)
"""Two-layer GCN (GraphConv norm='both') on 8 Trainium2 NeuronCores.

Strategy (graph/data parallel per the sharding hint):
  - dst nodes partitioned across 8 cores (6250 rows each); weights replicated.
  - The dense weight is COMMUTED through the segment-sum:
    sum_e ns[s]*nd[d]*(x[s] @ W) = (sum_e ns[s]*nd[d]*x[s]) @ W, so layer 1
    aggregates RAW x rows. Each core receives the full x as a plain input
    split into two HBM tables (no phase-A compute, no layer-1 collective:
    gathers start immediately). Layer 2 aggregates relu rows r = relu(agg1
    @ W1 + b1) and applies W2 + b2 after its segment-sum; only the r table
    needs the halo-exchange AllGather (one per layer boundary).
  - Edge aggregation per core: edges grouped by (dst-tile, table-half) and
    sorted by source; source rows fetched with SWDGE dma_gather (256B f16
    rows, 8-chunk single-packet windows rotating over 4 SWDGE queues);
    segment-sum on the PE as one-hot matmuls into PSUM [f_in x dst]. The
    edge-weight selector S (carrying ns*nd) is built ON-CHIP on the DVE:
    S = (iota == dstcol) * nrm from a tiny per-chunk metadata table.
  - int16 gather indices cover only 32768 rows, so tables are split at row
    32768 (part a = each core's rows 0..4095 concatenated, part b = the
    rest). 256B single-row gathers have zero fetch waste, and part-a
    gathers depend only on the part-a AllGather (Tile tracks DRAM deps per
    tensor), letting next-layer gathers start while part b is in flight.
  - Biases are rank-1 matmul updates (ones-column x bias-row) into the
    same PSUM tile as the dense matmul; per-tile flow is
    pa=[f,d] -> f16 -> @W -> [d,f] node-major (+bias) -> relu/copy.

All feature math runs on-device in fp16 (f32 PSUM accumulation); index
preprocessing (edge partitioning/sorting, degree counts, gather schedule)
is host-side sharding prep.
"""

import numpy as np

N_NODES = 50000
N_EDGES = 600000
D = 128
N_CORES = 8
NPC = N_NODES // N_CORES          # 6250 nodes per core
NT = (NPC + 127) // 128           # 49 dst tiles per core
RS = 4096                         # part-a rows per core (8*RS = 32768)
NA = N_CORES * RS                 # 32768 rows in table a
NB = N_NODES - NA                 # 17232 rows in table b
W = 8                             # gather window size (chunks per call)

_CACHE = {}


def _host_prep(x, src, dst, W1, b1, W2, b2):
    x = np.asarray(x, dtype=np.float32)
    src = np.asarray(src, dtype=np.int64)
    dst = np.asarray(dst, dtype=np.int64)
    W1 = np.asarray(W1, dtype=np.float32)
    W2 = np.asarray(W2, dtype=np.float32)
    b1 = np.asarray(b1, dtype=np.float32)
    b2 = np.asarray(b2, dtype=np.float32)

    deg_out = np.bincount(src, minlength=N_NODES).astype(np.float32)
    deg_in = np.bincount(dst, minlength=N_NODES).astype(np.float32)
    norm_src = np.where(deg_out > 0, 1.0 / np.sqrt(np.maximum(deg_out, 1.0)), 0.0)
    norm_dst = np.where(deg_in > 0, 1.0 / np.sqrt(np.maximum(deg_in, 1.0)), 0.0)
    norm_src = norm_src.astype(np.float32)
    norm_dst = norm_dst.astype(np.float32)

    # Map node -> row in the split-table layout: part a = each core's rows
    # [0, RS) concatenated; part b = rows [RS, NPC) concatenated.
    nodes = np.arange(N_NODES, dtype=np.int64)
    ksn, rsn = nodes // NPC, nodes % NPC
    HS = RS // 2                      # 2048 rows per core in each a-half
    NH = N_CORES * HS                 # 16384 rows per a-half table
    cat_of_node = np.where(
        rsn < HS, ksn * HS + rsn,
        np.where(rsn < RS, NH + ksn * HS + (rsn - HS),
                 NA + ksn * (NPC - RS) + (rsn - RS)))
    xcat = np.empty((N_NODES, D), dtype=np.float16)
    xcat[cat_of_node] = x.astype(np.float16)
    Xa1 = np.ascontiguousarray(xcat[:NH])
    Xa2 = np.ascontiguousarray(xcat[NH:NA])
    Xb = np.ascontiguousarray(xcat[NA:])

    cat = cat_of_node[src]
    half = np.where(cat < NH, 0, np.where(cat < NA, 1, 2))

    # --- per-core edge grouping by (dst tile, third), sorted by cat ---
    per_core = []
    cnts = np.zeros((N_CORES, NT, 3), dtype=np.int64)
    for k in range(N_CORES):
        m = (dst >= k * NPC) & (dst < (k + 1) * NPC)
        c_k = cat[m]
        s_k = src[m]
        dl_k = dst[m] - k * NPC
        t_k = dl_k >> 7
        h_k = half[m]
        key = t_k * 3 + h_k
        order = np.lexsort((c_k, key))
        per_core.append((c_k[order], s_k[order], dl_k[order], key[order],
                         h_k[order]))
        cnts[k] = np.bincount(key, minlength=NT * 3).reshape(NT, 3)

    # shared static schedule: chunks per (tile, third), max over cores
    C_s = [np.maximum.reduce([(cnts[k, :, j] + 127) // 128
                              for k in range(N_CORES)]) for j in range(3)]
    C_s[0] = np.where((C_s[0] + C_s[1] + C_s[2]) == 0, 1, C_s[0])
    base_s = [np.concatenate([[0], np.cumsum(C)[:-1]]) for C in C_s]
    n_s = [int(C.sum()) for C in C_s]
    cons_base = np.concatenate(
        [[0], np.cumsum(C_s[0] + C_s[1] + C_s[2])[:-1]])
    nch = sum(n_s)

    in_maps = []
    for k in range(N_CORES):
        c_k, s_k, dl_k, key, h_k = per_core[k]
        t_k = key // 3
        grp_counts = np.bincount(key, minlength=NT * 3)
        grp_start = np.concatenate([[0], np.cumsum(grp_counts)[:-1]])
        rank = np.arange(len(key)) - grp_start[key]
        chunk_in_grp = rank >> 7
        stream_base = np.select(
            [h_k == 0, h_k == 1],
            [base_s[0][t_k] * 128, base_s[1][t_k] * 128],
            base_s[2][t_k] * 128)
        pos = stream_base + rank

        rebase = np.select([h_k == 0, h_k == 1], [0, NH], NA)
        idxs = [np.zeros(n * 128, dtype=np.int16) for n in n_s]
        for j in range(3):
            mj = h_k == j
            idxs[j][pos[mj]] = (c_k[mj] - rebase[mj]).astype(np.int16)
        idx_l1, idx_l2, idx_hi = idxs

        # consumption order per tile: l1, l2, then hi chunks
        col = cons_base[t_k] + np.select(
            [h_k == 0, h_k == 1],
            [chunk_in_grp, C_s[0][t_k] + chunk_in_grp],
            C_s[0][t_k] + C_s[1][t_k] + chunk_in_grp)
        row = rank & 127
        ncp = (nch + W - 1) // W * W
        meta_dst = np.full((128, ncp), 999.0, dtype=np.float16)
        meta_nrm = np.zeros((128, ncp), dtype=np.float16)
        meta_dst[row, col] = (dl_k & 127).astype(np.float16)
        meta_nrm[row, col] = (norm_src[s_k]
                              * norm_dst[dl_k + k * NPC]).astype(np.float16)

        in_maps.append(
            {
                "Xa1": Xa1,
                "Xa2": Xa2,
                "Xb": Xb,
                "W1f": W1.astype(np.float16),
                "W2f": W2.astype(np.float16),
                "b1r": b1.reshape(1, 128).astype(np.float16),
                "b2r": b2.reshape(1, 128).astype(np.float16),
                "ones": np.ones((1, 128), dtype=np.float16),
                "iota8": np.tile(np.arange(128, dtype=np.float16), (128, W)),
                "idx_l1": np.tile(idx_l1.reshape(-1, 16).T, (8, 1)),
                "idx_l2": np.tile(idx_l2.reshape(-1, 16).T, (8, 1)),
                "idx_hi": np.tile(idx_hi.reshape(-1, 16).T, (8, 1)),
                "meta_dst": meta_dst,
                "meta_nrm": meta_nrm,
            }
        )
    sched = tuple(tuple(int(v) for v in C) for C in C_s)
    return in_maps, sched, nch


def _build_program(sched, nch):
    import concourse.bacc as bacc
    import concourse.mybir as mybir
    import concourse.tile as tile
    from concourse.library_config import mlp

    C_s = [np.array(C) for C in sched]
    base_s = [np.concatenate([[0], np.cumsum(C)[:-1]]).astype(int)
              for C in C_s]
    n_s = [int(C.sum()) for C in C_s]
    cons_base = np.concatenate(
        [[0], np.cumsum(C_s[0] + C_s[1] + C_s[2])[:-1]]).astype(int)
    ncp = (nch + W - 1) // W * W
    NH = N_CORES * (RS // 2)
    f16 = mybir.dt.float16
    f32 = mybir.dt.float32
    AF = mybir.ActivationFunctionType
    ALU = mybir.AluOpType

    nc = bacc.Bacc("TRN2", target_bir_lowering=False, debug=False,
                   num_devices=N_CORES, num_swdge_queues=4)

    Xa1_d = nc.dram_tensor("Xa1", [NH, D], f16, kind="ExternalInput")
    Xa2_d = nc.dram_tensor("Xa2", [NH, D], f16, kind="ExternalInput")
    Xb_d = nc.dram_tensor("Xb", [NB, D], f16, kind="ExternalInput")
    W1_d = nc.dram_tensor("W1f", [128, 128], f16, kind="ExternalInput")
    W2_d = nc.dram_tensor("W2f", [128, 128], f16, kind="ExternalInput")
    b1_d = nc.dram_tensor("b1r", [1, 128], f16, kind="ExternalInput")
    b2_d = nc.dram_tensor("b2r", [1, 128], f16, kind="ExternalInput")
    on_d = nc.dram_tensor("ones", [1, 128], f16, kind="ExternalInput")
    io_d = nc.dram_tensor("iota8", [128, W * 128], f16, kind="ExternalInput")
    il1_d = nc.dram_tensor("idx_l1", [128, n_s[0] * 8], mybir.dt.int16,
                           kind="ExternalInput")
    il2_d = nc.dram_tensor("idx_l2", [128, n_s[1] * 8], mybir.dt.int16,
                           kind="ExternalInput")
    ihi_d = nc.dram_tensor("idx_hi", [128, n_s[2] * 8], mybir.dt.int16,
                           kind="ExternalInput")
    mdst_d = nc.dram_tensor("meta_dst", [128, ncp], f16, kind="ExternalInput")
    mnrm_d = nc.dram_tensor("meta_nrm", [128, ncp], f16, kind="ExternalInput")

    rparts = (
        nc.dram_tensor("r_a1", [RS // 2, D], f16, kind="Internal"),
        nc.dram_tensor("r_a2", [RS // 2, D], f16, kind="Internal"),
        nc.dram_tensor("r_b", [NPC - RS, D], f16, kind="Internal"),
    )
    Ra1 = nc.dram_tensor("Ra1", [NH, D], f16, kind="Internal",
                         addr_space="Shared")
    Ra2 = nc.dram_tensor("Ra2", [NH, D], f16, kind="Internal",
                         addr_space="Shared")
    Rb = nc.dram_tensor("Rb", [NB, D], f16, kind="Internal",
                        addr_space="Shared")
    out_d = nc.dram_tensor("outN", [NT * 128, D], f32, kind="ExternalOutput")

    qctr = [0]

    def next_q():
        q = qctr[0] % 4
        qctr[0] += 1
        return q

    with tile.TileContext(nc) as tc:
        with (
            tc.tile_pool(name="consts", bufs=1) as consts,
            tc.tile_pool(name="mt", bufs=24) as mt_pool,
            tc.tile_pool(name="st", bufs=16) as st_pool,
            tc.tile_pool(name="hb", bufs=6) as hb_pool,
            tc.tile_pool(name="pf", bufs=4) as pf_pool,
            tc.tile_pool(name="psa", bufs=3, space="PSUM") as psa_pool,
            tc.tile_pool(name="ps", bufs=5, space="PSUM") as ps_pool,
        ):
            nc.gpsimd.load_library(mlp)

            W1f = consts.tile([128, 128], f16, tag="W1f")
            W2f = consts.tile([128, 128], f16, tag="W2f")
            b1r = consts.tile([1, 128], f16, tag="b1r")
            b2r = consts.tile([1, 128], f16, tag="b2r")
            ones = consts.tile([1, 128], f16, tag="ones")
            iota8 = consts.tile([128, W, 128], f16, tag="iota8")
            idx_l1 = consts.tile([128, n_s[0] * 8], mybir.dt.int16, tag="il1")
            idx_l2 = consts.tile([128, n_s[1] * 8], mybir.dt.int16, tag="il2")
            idx_hi = consts.tile([128, n_s[2] * 8], mybir.dt.int16, tag="ihi")
            mdst = consts.tile([128, ncp], f16, tag="mdst")
            mnrm = consts.tile([128, ncp], f16, tag="mnrm")
            nc.sync.dma_start(idx_l1[:], il1_d.ap())
            nc.sync.dma_start(mdst[:], mdst_d.ap())
            nc.sync.dma_start(mnrm[:], mnrm_d.ap())
            nc.sync.dma_start(idx_l2[:], il2_d.ap())
            nc.sync.dma_start(idx_hi[:], ihi_d.ap())
            nc.sync.dma_start(W1f[:], W1_d.ap())
            nc.sync.dma_start(W2f[:], W2_d.ap())
            nc.sync.dma_start(b1r[:], b1_d.ap())
            nc.sync.dma_start(b2r[:], b2_d.ap())
            nc.sync.dma_start(ones[:], on_d.ap())
            nc.sync.dma_start(iota8[:],
                              io_d.ap().rearrange("p (a e) -> p a e", a=W))

            BT = 4
            TSPLIT = RS // 128      # 32

            def make_h_writer(h_dram, t_lo, t_hi):
                """Write tiles [t_lo, t_hi) into h_dram (rows rebased)."""
                nfull = min(t_hi, NPC // 128) - t_lo
                h3 = h_dram.ap()[0 : nfull * 128, :].rearrange(
                    "(a p) d -> p a d", p=128
                )
                state = {}

                def write(t, produce):
                    tl_ = t - t_lo
                    if tl_ < nfull:
                        g = tl_ - tl_ % BT
                        if tl_ % BT == 0:
                            state["buf"] = hb_pool.tile(
                                [128, BT, 128], f16, tag="hstage", name="hstage"
                            )
                        produce(state["buf"][:, tl_ % BT, :])
                        if tl_ % BT == BT - 1 or tl_ == nfull - 1:
                            n = tl_ - g + 1
                            nc.scalar.dma_start(h3[:, g : g + n, :],
                                                state["buf"][:, 0:n, :])
                    else:
                        rows = NPC - t * 128
                        tl = hb_pool.tile([128, 128], f16, tag="hrag",
                                          name="hrag")
                        produce(tl[:])
                        nc.scalar.dma_start(
                            h_dram.ap()[tl_ * 128 : tl_ * 128 + rows, :],
                            tl[:rows, :],
                        )

                return write

            def sub_allgather(h_sub, H_out):
                nc.gpsimd.collective_compute(
                    "AllGather", mybir.AluOpType.bypass,
                    replica_groups=[list(range(N_CORES))],
                    ins=[h_sub.ap()], outs=[H_out.ap()],
                )

            def agg_phase(Ta1, Ta2, Tb, out_cb, pre_loop=None):
                """Gather in fixed W-chunk windows per stream; per dst tile
                accumulate segment-sum matmuls into psum [f x d], then
                out_cb(t, pa). Selector built on DVE per S-window."""
                streams = {0: (Ta1, idx_l1, n_s[0]), 1: (Ta2, idx_l2, n_s[1]),
                           2: (Tb, idx_hi, n_s[2])}
                mt_tiles = {}
                st_tiles = {}

                def ensure_window(s, w):
                    key = (s, w)
                    if key in mt_tiles:
                        return
                    tab, idx_t, n_s = streams[s]
                    cb = w * W
                    cw = min(W, n_s - cb)
                    mt = mt_pool.tile([128, cw, 128], f16, tag="mt")
                    nc.gpsimd.dma_gather(
                        mt[:], tab.ap(),
                        idx_t[:, cb * 8 : (cb + cw) * 8],
                        cw * 128, cw * 128, 128,
                        queue_num=next_q(),
                    )
                    mt_tiles[key] = mt

                def ensure_s(cons):
                    sw, so = cons // W, cons % W
                    if sw not in st_tiles:
                        st = st_pool.tile([128, W, 128], f16, tag="st")
                        md = mdst[:, sw * W : (sw + 1) * W].rearrange(
                            "p (a b) -> p a b", b=1).to_broadcast([128, W, 128])
                        mn = mnrm[:, sw * W : (sw + 1) * W].rearrange(
                            "p (a b) -> p a b", b=1).to_broadcast([128, W, 128])
                        nc.vector.tensor_tensor(out=st[:], in0=iota8[:],
                                                in1=md, op=ALU.is_equal)
                        nc.vector.tensor_tensor(out=st[:], in0=st[:],
                                                in1=mn, op=ALU.mult)
                        st_tiles[sw] = st
                    return st_tiles[sw][:, so, :]

                # Prefetch early-stream windows so the in-order Pool engine
                # has queued DMA work to drain while later streams wait on
                # their tables' AllGathers.
                for w in range(min(14, (n_s[0] + W - 1) // W)):
                    ensure_window(0, w)
                for w in range(min(6, (n_s[1] + W - 1) // W)):
                    ensure_window(1, w)
                if pre_loop is not None:
                    pre_loop()

                def chunk_list(t):
                    out = []
                    for j in range(3):
                        for i in range(C_s[j][t]):
                            out.append((j, int(base_s[j][t]) + i))
                    return out

                pending = None
                for t in range(NT):
                    # Emit tile t+2's gather windows and S-builds ahead so
                    # that by the time their matmuls issue, every semaphore
                    # is already satisfied (fast-path dispatch on the PE).
                    if t + 2 < NT:
                        for i2, (s2, sc2) in enumerate(chunk_list(t + 2)):
                            ensure_window(s2, sc2 // W)
                            ensure_s(int(cons_base[t + 2]) + i2)
                    chunks = chunk_list(t)
                    pa = ps_pool.tile([128, 128], f32, tag="pa")
                    for i, (s, sc) in enumerate(chunks):
                        w, o = sc // W, sc % W
                        ensure_window(s, w)
                        cons = int(cons_base[t]) + i
                        s_ap = ensure_s(cons)
                        nc.tensor.matmul(
                            pa[:], mt_tiles[(s, w)][:, o, :], s_ap,
                            start=(i == 0), stop=(i == len(chunks) - 1),
                        )
                    # Defer the per-tile chain one tile: its cross-engine
                    # round-trips (psum copy -> W matmul -> relu/out) then
                    # overlap tile t+1's chunk matmuls instead of stalling
                    # the in-order PE queue.
                    if pending is not None:
                        out_cb(pending[0], pending[1])
                    pending = (t, pa)
                out_cb(pending[0], pending[1])

            # ---- layer 1: agg(x) -> @W1 + b1 -> relu -> r parts + AGs ----
            TS1 = TSPLIT // 2       # 16
            w1_a = make_h_writer(rparts[0], 0, TS1)
            w1_b = make_h_writer(rparts[1], TS1, TSPLIT)
            w1_c = make_h_writer(rparts[2], TSPLIT, NT)

            def phase_1(t, pa):
                paf = pf_pool.tile([128, 128], f16, tag="paf")
                nc.scalar.activation(paf[:], pa[:], AF.Copy)
                ph = psa_pool.tile([128, 128], f32, tag="ph", name="ph")
                nc.tensor.matmul(ph[:], paf[:], W1f[:], start=True, stop=False)
                nc.tensor.matmul(ph[:], ones[:], b1r[:], start=False, stop=True)
                wsel = w1_a if t < TS1 else (w1_b if t < TSPLIT else w1_c)
                wsel(t, lambda dst, ph=ph: nc.scalar.activation(
                    dst, ph[:], AF.Relu))
                # Trigger each part's AllGather a couple of tiles after its
                # last h-write so the in-order Pool engine reaches the
                # blocking trigger just as the writes it waits on complete.
                if t == TS1 + 1:
                    sub_allgather(rparts[0], Ra1)
                if t == TSPLIT + 1:
                    sub_allgather(rparts[1], Ra2)

            agg_phase(Xa1_d, Xa2_d, Xb_d, phase_1)

            # ---- layer 2: agg(r) -> @W2 + b2 -> out (node-major) ----
            o3 = out_d.ap().rearrange("(a p) d -> p a d", p=128)
            ostate = {}

            def phase_2(t, pa):
                paf = pf_pool.tile([128, 128], f16, tag="paf")
                nc.scalar.activation(paf[:], pa[:], AF.Copy)
                ph = psa_pool.tile([128, 128], f32, tag="ph", name="ph2")
                nc.tensor.matmul(ph[:], paf[:], W2f[:], start=True, stop=False)
                nc.tensor.matmul(ph[:], ones[:], b2r[:], start=False, stop=True)
                g = t - t % BT
                if t % BT == 0:
                    ostate["buf"] = hb_pool.tile([128, BT, 128], f32,
                                                 tag="ostage", name="ostage")
                nc.scalar.activation(ostate["buf"][:, t % BT, :], ph[:],
                                     AF.Copy)
                if t % BT == BT - 1 or t == NT - 1:
                    n = t - g + 1
                    nc.scalar.dma_start(o3[:, g : g + n, :],
                                        ostate["buf"][:, 0:n, :])

            # The part-b AllGather trigger is emitted after layer 2's lo
            # prefetch so those window issues queue ahead of the blocking
            # trigger on the in-order Pool engine.
            agg_phase(Ra1, Ra2, Rb, phase_2,
                      pre_loop=lambda: sub_allgather(rparts[2], Rb))

    nc.compile()
    return nc


def kernel(x, src, dst, W1, b1, W2, b2):
    from concourse.bass_utils import run_bass_kernel_spmd

    in_maps, sched, nch = _host_prep(x, src, dst, W1, b1, W2, b2)
    key = (sched, nch)
    if key not in _CACHE:
        _CACHE[key] = _build_program(sched, nch)
    nc = _CACHE[key]
    res = run_bass_kernel_spmd(nc, in_maps, core_ids=list(range(N_CORES)))
    out = np.empty((N_NODES, D), dtype=np.float32)
    for k in range(N_CORES):
        out[k * NPC : (k + 1) * NPC] = res.results[k]["outN"][:NPC]
    return out
